# revision 5
# baseline (speedup 1.0000x reference)
"""Trainium2 Bass kernel: GPT-2 style causal attention + output projection.

Reference computation (B=2, L=2048, D=1024, H=16, dh=64):
    q,k,v = split_heads(query/key/value)            # [B,H,L,dh]
    S = q @ k^T / sqrt(dh)                          # [B,H,L,L]
    P = softmax(causal_mask(S))
    A = merge_heads(P @ v)                          # [B,L,D]
    out = A @ w_proj + b_proj

Sharding: 32 (b,h) pairs, 4 per core (cores 0-3 batch 0, 4-7 batch 1).
Each core computes attention for its 4 heads fully causally and a partial
c_proj using its 256 rows of w_proj; the host sums the 4 partials per batch.

Device-side layout trick: scores are computed transposed (S^T, keys on
partitions) so softmax's P lands with keys on the partition axis, which is
exactly the contraction layout P.V needs -- no on-device transposes anywhere.
A ones-column appended to V makes the same matmul emit softmax denominators.
"""

import numpy as np

B, L, D, H = 2, 2048, 1024, 16
DH = 64          # head dim
PAIRS = 4        # (b,h) pairs per core
QB = 512         # query block
KC = 128         # key chunk
NCORES = 8

_COMPILED = None  # (nc, )


def _build_nc():
    import concourse.bacc as bacc
    import concourse.tile as tile
    from concourse import mybir

    f32 = mybir.dt.float32
    f32r = mybir.dt.float32r
    Exp = mybir.ActivationFunctionType.Exp

    nc = bacc.Bacc("TRN2", target_bir_lowering=False, debug=False,
                   num_devices=NCORES)

    qt_d = nc.dram_tensor("qt", [2, 128, L], f32r, kind="ExternalInput").ap()
    kt_d = nc.dram_tensor("kt", [2, 128, L], f32r, kind="ExternalInput").ap()
    v_d = nc.dram_tensor("v", [PAIRS, L, DH + 1], f32r, kind="ExternalInput").ap()
    masks_d = nc.dram_tensor("masks", [128, 4 * 1024], f32, kind="ExternalInput").ap()
    w_d = nc.dram_tensor("w", [2, 128, D], f32r, kind="ExternalInput").ap()
    ones_d = nc.dram_tensor("ones", [1, DH], f32r, kind="ExternalInput").ap()
    out_d = nc.dram_tensor("out", [L, D], f32, kind="ExternalOutput").ap()

    with tile.TileContext(nc) as tc:
        with (
            tc.tile_pool(name="consts", bufs=1) as consts,
            tc.tile_pool(name="st", bufs=2, space="PSUM") as st_pool,
            tc.tile_pool(name="at", bufs=2, space="PSUM") as at_pool,
            tc.tile_pool(name="bc", bufs=1, space="PSUM") as bc_pool,
            tc.tile_pool(name="cp", bufs=1, space="PSUM") as cp_pool,
            tc.tile_pool(name="et", bufs=3) as et_pool,
            tc.tile_pool(name="atn", bufs=4) as atn_pool,
            tc.tile_pool(name="rec", bufs=2) as rec_pool,
            tc.tile_pool(name="bcs", bufs=2) as bcs_pool,
            tc.tile_pool(name="osb", bufs=2) as osb_pool,
        ):
            # resident inputs
            qt = [consts.tile([128, L], f32r, name=f"qt{i}", tag=f"qt{i}") for i in range(2)]
            kt = [consts.tile([128, L], f32r, name=f"kt{i}", tag=f"kt{i}") for i in range(2)]
            vt = [consts.tile([128, (L // KC) * (DH + 1)], f32r, name=f"vt{i}", tag=f"vt{i}")
                  for i in range(PAIRS)]
            mk = consts.tile([128, 4 * 1024], f32, name="mk", tag="mk")
            wt = [consts.tile([128, D], f32r, name=f"wt{i}", tag=f"wt{i}") for i in range(2)]
            ones = consts.tile([1, DH], f32r, name="ones", tag="ones")

            for i in range(2):
                nc.sync.dma_start(qt[i][:], qt_d[i])
                nc.sync.dma_start(kt[i][:], kt_d[i])
                nc.sync.dma_start(wt[i][:], w_d[i])
            for p in range(PAIRS):
                nc.sync.dma_start(
                    vt[p][:].rearrange("p (c d) -> p c d", d=DH + 1),
                    v_d[p].rearrange("(c p) d -> p c d", p=128),
                )
            nc.sync.dma_start(mk[:], masks_d[:])
            nc.sync.dma_start(ones[:], ones_d[:])

            for J in range(L // QB):
                nch = 4 * J + 4          # causal: key chunks 0..nch-1
                atn_duo = []
                for duo in range(2):
                    at = [at_pool.tile([65, QB], f32, name="at", tag="at") for _ in range(2)]
                    for c in range(nch):
                        st = st_pool.tile([128, 2 * QB], f32)
                        for h2 in range(2):
                            nc.tensor.matmul(
                                st[:, h2 * QB:(h2 + 1) * QB],
                                lhsT=kt[duo][64 * h2:64 * (h2 + 1),
                                             c * KC:(c + 1) * KC],
                                rhs=qt[duo][64 * h2:64 * (h2 + 1),
                                            J * QB:(J + 1) * QB],
                                start=True, stop=True,
                                tile_position=(64 * h2, 0),
                            )
                        et = et_pool.tile([128, 2 * QB], f32r)
                        nc.scalar.activation(et[:], st[:], Exp, scale=0.125)
                        m = c - 4 * J
                        if m >= 0:
                            nc.vector.tensor_mul(
                                et[:], et[:], mk[:, m * 1024:(m + 1) * 1024])
                        for h2 in range(2):
                            pair = 2 * duo + h2
                            nc.tensor.matmul(
                                at[h2][0:65, :],
                                lhsT=vt[pair][:, c * (DH + 1):(c + 1) * (DH + 1)
                                              ],
                                rhs=et[:, h2 * QB:(h2 + 1) * QB],
                                start=(c == 0), stop=(c == nch - 1),
                            )
                    atn = atn_pool.tile([128, QB], f32r)
                    for h2 in range(2):
                        rec = rec_pool.tile([1, QB], f32r)
                        with nc.allow_low_precision(reason="softmax recip rounds to fp32r"):
                            nc.vector.reciprocal(rec[:], at[h2][64:65, :])
                        bc = bc_pool.tile([64, QB], f32)
                        nc.tensor.matmul(
                            bc[:], lhsT=ones[:],
                            rhs=rec[:], start=True, stop=True)
                        bcs = bcs_pool.tile([64, QB], f32, name="bcs", tag="bcs")
                        nc.vector.tensor_copy(bcs[:], bc[:])
                        nc.vector.tensor_mul(
                            atn[64 * h2:64 * (h2 + 1), :], at[h2][0:64, :], bcs[:])
                    atn_duo.append(atn)

                # partial c_proj for this J's 512 rows
                for rt in range(QB // 128):
                    for nf in range(2):
                        cp = cp_pool.tile([128, 512], f32)
                        for duo in range(2):
                            nc.tensor.matmul(
                                cp[:],
                                lhsT=atn_duo[duo][:, rt * 128:(rt + 1) * 128
                                                  ],
                                rhs=wt[duo][:, nf * 512:(nf + 1) * 512
                                            ],
                                start=(duo == 0), stop=(duo == 1),
                            )
                        ob = osb_pool.tile([128, 512], f32)
                        nc.vector.tensor_copy(ob[:], cp[:])
                        nc.sync.dma_start(
                            out_d[J * QB + rt * 128:J * QB + (rt + 1) * 128,
                                  nf * 512:(nf + 1) * 512],
                            ob[:],
                        )

    nc.compile()
    return nc


def _get_nc():
    global _COMPILED
    if _COMPILED is None:
        _COMPILED = _build_nc()
    return _COMPILED


def kernel(query, key, value, w_proj, b_proj, n_head):
    from concourse.bass_utils import run_bass_kernel_spmd

    q = np.ascontiguousarray(np.asarray(query, dtype=np.float32))
    k = np.ascontiguousarray(np.asarray(key, dtype=np.float32))
    v = np.ascontiguousarray(np.asarray(value, dtype=np.float32))
    w = np.ascontiguousarray(np.asarray(w_proj, dtype=np.float32))
    bias = np.asarray(b_proj, dtype=np.float32)

    q4 = q.reshape(B, L, H, DH)
    k4 = k.reshape(B, L, H, DH)
    v4 = v.reshape(B, L, H, DH)

    kp = np.arange(128)[:, None]
    qf = np.arange(QB)[None, :]
    mk_parts = []
    for m in range(4):
        mm = (kp + 128 * m <= qf).astype(np.float32)        # [128, 512]
        mk_parts.append(np.concatenate([mm, mm], axis=1))    # [128, 1024]
    masks = np.ascontiguousarray(np.concatenate(mk_parts, axis=1))  # [128,4096]
    ones64 = np.ones((1, DH), dtype=np.float32)

    in_maps = []
    for c in range(NCORES):
        b = c // 4
        hsel = 4 * (c % 4)
        qb_t = q4[b].transpose(1, 2, 0)   # [H, DH, L]
        kb_t = k4[b].transpose(1, 2, 0)
        qt = np.ascontiguousarray(
            qb_t[hsel:hsel + 4].reshape(2, 128, L))
        kt = np.ascontiguousarray(
            kb_t[hsel:hsel + 4].reshape(2, 128, L))
        vsl = v4[b, :, hsel:hsel + 4, :].transpose(1, 0, 2)  # [4, L, DH]
        vext = np.concatenate(
            [vsl, np.ones((PAIRS, L, 1), dtype=np.float32)], axis=2)
        vext = np.ascontiguousarray(vext)
        wp = np.ascontiguousarray(
            w[(c % 4) * 256:(c % 4 + 1) * 256, :].reshape(2, 128, D))
        in_maps.append({"qt": qt, "kt": kt, "v": vext, "masks": masks,
                        "w": wp, "ones": ones64})

    nc = _get_nc()
    res = run_bass_kernel_spmd(nc, in_maps, list(range(NCORES)))

    out = np.zeros((B, L, D), dtype=np.float32)
    for c in range(NCORES):
        out[c // 4] += res.results[c]["out"]
    out += bias[None, None, :]
    return out


# revision 6
# speedup vs baseline: 1.4156x; 1.4156x over previous
"""Trainium2 Bass kernel: GPT-2 style causal attention + output projection.

Reference computation (B=2, L=2048, D=1024, H=16, dh=64):
    q,k,v = split_heads(query/key/value)            # [B,H,L,dh]
    S = q @ k^T / sqrt(dh)                          # [B,H,L,L]
    P = softmax(causal_mask(S))
    A = merge_heads(P @ v)                          # [B,L,D]
    out = A @ w_proj + b_proj

Sharding: 32 (b,h) pairs, 4 per core (cores 0-3 batch 0, 4-7 batch 1).
Each core computes attention for its 4 heads fully causally and a partial
c_proj using its 256 rows of w_proj; the host sums the 4 partials per batch.

Device-side layout trick: scores are computed transposed (S^T, keys on
partitions) so softmax's P lands with keys on the partition axis, which is
exactly the contraction layout P.V needs -- no on-device transposes anywhere.
A ones-column appended to V makes the same matmul emit softmax denominators.

Precision: QK^T in fp32r (fp22 single-pass); probabilities/V/c_proj in bf16
with fp32 PSUM accumulation; softmax normalization in fp32.
"""

import numpy as np

B, L, D, H = 2, 2048, 1024, 16
DH = 64          # head dim
PAIRS = 4        # (b,h) pairs per core
QB = 512         # query block
KC = 128         # key chunk
NCORES = 8

_COMPILED = None


def _build_nc():
    import concourse.bacc as bacc
    import concourse.tile as tile
    from concourse import mybir

    f32 = mybir.dt.float32
    f32r = mybir.dt.float32r
    bf16 = mybir.dt.bfloat16
    Exp = mybir.ActivationFunctionType.Exp

    nc = bacc.Bacc("TRN2", target_bir_lowering=False, debug=False,
                   num_devices=NCORES)

    qt_d = nc.dram_tensor("qt", [2, 128, L], f32r, kind="ExternalInput").ap()
    kt_d = nc.dram_tensor("kt", [2, 128, L], f32r, kind="ExternalInput").ap()
    v_d = nc.dram_tensor("v", [PAIRS, L, DH + 1], bf16, kind="ExternalInput").ap()
    masks_d = nc.dram_tensor("masks", [128, 4 * 1024], bf16, kind="ExternalInput").ap()
    w_d = nc.dram_tensor("w", [2, 128, D], bf16, kind="ExternalInput").ap()
    ones_d = nc.dram_tensor("ones", [1, DH], f32r, kind="ExternalInput").ap()
    out_d = nc.dram_tensor("out", [L, D], f32, kind="ExternalOutput").ap()

    with tile.TileContext(nc) as tc:
        with (
            tc.tile_pool(name="consts", bufs=1) as consts,
            tc.tile_pool(name="st", bufs=2, space="PSUM") as st_pool,
            tc.tile_pool(name="at", bufs=2, space="PSUM") as at_pool,
            tc.tile_pool(name="bc", bufs=1, space="PSUM") as bc_pool,
            tc.tile_pool(name="cp", bufs=1, space="PSUM") as cp_pool,
            tc.tile_pool(name="et", bufs=3) as et_pool,
            tc.tile_pool(name="atn", bufs=4) as atn_pool,
            tc.tile_pool(name="dsb", bufs=2) as dsb_pool,
            tc.tile_pool(name="rbc", bufs=2) as rbc_pool,
            tc.tile_pool(name="osb", bufs=3) as osb_pool,
        ):
            # resident inputs
            qt = [consts.tile([128, L], f32r, name=f"qt{i}", tag=f"qt{i}")
                  for i in range(2)]
            kt = [consts.tile([128, L], f32r, name=f"kt{i}", tag=f"kt{i}")
                  for i in range(2)]
            vt = [consts.tile([128, (L // KC) * (DH + 1)], bf16,
                              name=f"vt{i}", tag=f"vt{i}") for i in range(PAIRS)]
            mk = consts.tile([128, 4 * 1024], bf16, name="mk", tag="mk")
            wt = [consts.tile([128, D], bf16, name=f"wt{i}", tag=f"wt{i}")
                  for i in range(2)]
            ones = consts.tile([1, DH], f32r, name="ones", tag="ones")

            # order: first compute needs kt0/qt0/masks, then the rest
            nc.sync.dma_start(kt[0][:], kt_d[0])
            nc.sync.dma_start(qt[0][:], qt_d[0])
            nc.sync.dma_start(mk[:], masks_d[:])
            nc.sync.dma_start(ones[:], ones_d[:])
            nc.sync.dma_start(kt[1][:], kt_d[1])
            nc.sync.dma_start(qt[1][:], qt_d[1])
            for p in range(PAIRS):
                nc.sync.dma_start(
                    vt[p][:].rearrange("p (c d) -> p c d", d=DH + 1),
                    v_d[p].rearrange("(c p) d -> p c d", p=128),
                )
            for i in range(2):
                nc.sync.dma_start(wt[i][:], w_d[i])

            for J in range(L // QB):
                nch = 4 * J + 4          # causal: key chunks 0..nch-1
                atn_duo = []
                for duo in range(2):
                    at = [at_pool.tile([65, QB], f32, name="at", tag="at")
                          for _ in range(2)]
                    for c in range(nch):
                        st = st_pool.tile([128, 2 * QB], f32, name="st",
                                          tag="st")
                        for h2 in range(2):
                            nc.tensor.matmul(
                                st[:, h2 * QB:(h2 + 1) * QB],
                                lhsT=kt[duo][64 * h2:64 * (h2 + 1),
                                             c * KC:(c + 1) * KC],
                                rhs=qt[duo][64 * h2:64 * (h2 + 1),
                                            J * QB:(J + 1) * QB],
                                start=True, stop=True,
                                tile_position=(64 * h2, 0),
                            )
                        et = et_pool.tile([128, 2 * QB], bf16, name="et",
                                          tag="et")
                        m = c - 4 * J
                        if m >= 1 and J > 0:
                            # diagonal chunk: only q >= m*128 within each pair
                            # half is causally valid (mask zeroes the stale
                            # rest; off at J==0 so each et slot is written
                            # fully once before any partial write)
                            nc.scalar.activation(
                                et[:].rearrange("p (h q) -> p h q", h=2)
                                [:, :, m * KC:],
                                st[:].rearrange("p (h q) -> p h q", h=2)
                                [:, :, m * KC:],
                                Exp, scale=0.125)
                        else:
                            nc.scalar.activation(et[:], st[:], Exp, scale=0.125)
                        if m >= 0:
                            nc.vector.tensor_mul(
                                et[:], et[:], mk[:, m * 1024:(m + 1) * 1024])
                        for h2 in range(2):
                            pair = 2 * duo + h2
                            nc.tensor.matmul(
                                at[h2][0:65, :],
                                lhsT=vt[pair][:, c * (DH + 1):(c + 1) * (DH + 1)],
                                rhs=et[:, h2 * QB:(h2 + 1) * QB],
                                start=(c == 0), stop=(c == nch - 1),
                            )
                    atn = atn_pool.tile([128, QB], bf16, name="atn", tag="atn")
                    for h2 in range(2):
                        dsb = dsb_pool.tile([1, QB], f32r, name="dsb", tag="dsb")
                        nc.vector.tensor_copy(dsb[:], at[h2][64:65, :])
                        bc = bc_pool.tile([64, QB], f32, name="bc", tag="bc")
                        nc.tensor.matmul(bc[:], lhsT=ones[:], rhs=dsb[:],
                                         start=True, stop=True)
                        rbc = rbc_pool.tile([64, QB], f32, name="rbc", tag="rbc")
                        nc.vector.reciprocal_approx_fast(rbc[:], bc[:])
                        nc.vector.tensor_mul(
                            atn[64 * h2:64 * (h2 + 1), :], at[h2][0:64, :],
                            rbc[:])
                    atn_duo.append(atn)

                # partial c_proj for this J's 512 rows
                for rt in range(QB // 128):
                    for nf in range(2):
                        cp = cp_pool.tile([128, 512], f32, name="cp", tag="cp")
                        for duo in range(2):
                            nc.tensor.matmul(
                                cp[:],
                                lhsT=atn_duo[duo][:, rt * 128:(rt + 1) * 128],
                                rhs=wt[duo][:, nf * 512:(nf + 1) * 512],
                                start=(duo == 0), stop=(duo == 1),
                            )
                        ob = osb_pool.tile([128, 512], f32, name="ob", tag="ob")
                        nc.vector.tensor_copy(ob[:], cp[:])
                        nc.sync.dma_start(
                            out_d[J * QB + rt * 128:J * QB + (rt + 1) * 128,
                                  nf * 512:(nf + 1) * 512],
                            ob[:],
                        )

    nc.compile()
    return nc


def _get_nc():
    global _COMPILED
    if _COMPILED is None:
        _COMPILED = _build_nc()
    return _COMPILED


def _prep_in_maps(query, key, value, w_proj):
    import ml_dtypes

    q = np.asarray(query, dtype=np.float32)
    k = np.asarray(key, dtype=np.float32)
    v = np.asarray(value, dtype=np.float32)
    w = np.asarray(w_proj, dtype=np.float32)

    q4 = q.reshape(B, L, H, DH)
    k4 = k.reshape(B, L, H, DH)
    v4 = v.reshape(B, L, H, DH)

    kp = np.arange(128)[:, None]
    qf = np.arange(QB)[None, :]
    mk_parts = []
    for m in range(4):
        mm = (kp + 128 * m <= qf).astype(np.float32)        # [128, 512]
        mk_parts.append(np.concatenate([mm, mm], axis=1))    # [128, 1024]
    masks = np.ascontiguousarray(
        np.concatenate(mk_parts, axis=1).astype(ml_dtypes.bfloat16))
    ones64 = np.ones((1, DH), dtype=np.float32)

    in_maps = []
    for c in range(NCORES):
        b = c // 4
        hsel = 4 * (c % 4)
        qt = np.ascontiguousarray(
            q4[b].transpose(1, 2, 0)[hsel:hsel + 4].reshape(2, 128, L))
        kt = np.ascontiguousarray(
            k4[b].transpose(1, 2, 0)[hsel:hsel + 4].reshape(2, 128, L))
        vsl = v4[b, :, hsel:hsel + 4, :].transpose(1, 0, 2)  # [4, L, DH]
        vext = np.concatenate(
            [vsl, np.ones((PAIRS, L, 1), dtype=np.float32)], axis=2)
        vext = np.ascontiguousarray(vext.astype(ml_dtypes.bfloat16))
        wp = np.ascontiguousarray(
            w[(c % 4) * 256:(c % 4 + 1) * 256, :].reshape(2, 128, D)
            .astype(ml_dtypes.bfloat16))
        in_maps.append({"qt": qt, "kt": kt, "v": vext, "masks": masks,
                        "w": wp, "ones": ones64})
    return in_maps


def kernel(query, key, value, w_proj, b_proj, n_head):
    from concourse.bass_utils import run_bass_kernel_spmd

    bias = np.asarray(b_proj, dtype=np.float32)
    in_maps = _prep_in_maps(query, key, value, w_proj)
    nc = _get_nc()
    res = run_bass_kernel_spmd(nc, in_maps, list(range(NCORES)))

    out = np.zeros((B, L, D), dtype=np.float32)
    for c in range(NCORES):
        out[c // 4] += res.results[c]["out"]
    out += bias[None, None, :]
    return out
